# revision 2
# baseline (speedup 1.0000x reference)
import time
import numpy as np
import jax
import jax.numpy as jnp

# Problem constants (hardcoded per spec nn_ActorNetwork_8031588844054)
BS = 256; NUPG = 300; NSPG = 30; FIN = 32; C = 128; H = 2; D = 64; L = 4; K = 2
NSH = 8                      # 8 NeuronCores, data-parallel over graphs
G = BS // NSH                # 32 graphs per shard
NU_S = G * NUPG; NS_S = G * NSPG; ND_S = G * NSPG
REL = ((0, 1), (1, 0), (0, 2), (2, 0), (2, 1), (1, 2))
SQD = float(np.sqrt(D))
NEG = -1e9

_E_PER_G = (NUPG, NUPG, NUPG * K, NUPG * K, NSPG, NSPG)  # edges per graph per relation
_TYPE_SHARD = (NU_S, NS_S, NS_S)  # per-type nodes per shard
_TYPE_PG = (NUPG, NSPG, NSPG)


def _dense_soft(Lcat, Mcat, Vcat):
    """Joint masked softmax over the last axis + weighted message aggregation.

    Lcat [G,H,N,Ct] logits, Mcat [G,N,Ct] edge multiplicities (0/1/2),
    Vcat [G,Ct,H,D] messages. Returns agg [G,N,H,D].
    Matches reference segment softmax: max over valid edges, exp, weighted sum;
    multiplicity m contributes m identical edges.
    """
    Mh = Mcat[:, None]                                  # [G,1,N,Ct]
    addm = jnp.where(Mh > 0, 0.0, NEG)
    am = jnp.max(Lcat + addm, axis=-1, keepdims=True)
    ex = Mh * jnp.exp(Lcat - am)
    den = jnp.sum(ex, axis=-1, keepdims=True)
    w = ex / jnp.maximum(den, 1e-16)                    # [G,H,N,Ct]
    return jnp.einsum('ghnc,gche->gnhe', w, Vcat)


def _hgt_layer(xs, kw, kb, qw, qb, vw, vb, ar, mr, pr, ow, ob, sk,
               M0, M1, M2, M3, P4, P5):
    k = [jnp.reshape(xs[t] @ kw[t] + kb[t], (-1, H, D)) for t in range(3)]
    q = [jnp.reshape(xs[t] @ qw[t] + qb[t], (-1, H, D)) for t in range(3)]
    v = [jnp.reshape(xs[t] @ vw[t] + vb[t], (-1, H, D)) for t in range(3)]
    kr = [jnp.einsum('nhd,hde->nhe', k[REL[r][0]], ar[r]).reshape(G, _TYPE_PG[REL[r][0]], H, D)
          for r in range(6)]
    vr = [jnp.einsum('nhd,hde->nhe', v[REL[r][0]], mr[r]).reshape(G, _TYPE_PG[REL[r][0]], H, D)
          for r in range(6)]
    qg = [q[t].reshape(G, _TYPE_PG[t], H, D) for t in range(3)]
    sc = [pr[r] / SQD for r in range(6)]                # [H] per relation

    # units (type 0): incoming rel1 (src-servers) + rel3 (dst-servers)
    L1 = jnp.einsum('guhd,gshd->ghus', qg[0], kr[1]) * sc[1][None, :, None, None]
    L3 = jnp.einsum('guhd,gshd->ghus', qg[0], kr[3]) * sc[3][None, :, None, None]
    Lu = jnp.concatenate([L1, L3], -1)
    Mu = jnp.concatenate([M1, M3], -1)
    Vu = jnp.concatenate([vr[1], vr[3]], 1)
    agg_u = _dense_soft(Lu, Mu, Vu).reshape(NU_S, C)

    # src-servers (type 1): incoming rel0 (units) + rel4 (paired dst-server)
    kr4p = jnp.einsum('gst,gthd->gshd', P4, kr[4])
    vr4p = jnp.einsum('gst,gthd->gshd', P4, vr[4])
    L0 = jnp.einsum('gshd,guhd->ghsu', qg[1], kr[0]) * sc[0][None, :, None, None]
    a4 = jnp.einsum('gshd,gshd->ghs', qg[1], kr4p) * sc[4][None, :, None]
    Ls = jnp.concatenate([L0, a4[..., None]], -1)
    Ms = jnp.concatenate([M0, jnp.ones((G, NSPG, 1), M0.dtype)], -1)
    Vs = jnp.concatenate([vr[0], jnp.zeros((G, 1, H, D), vr[0].dtype)], 1)
    agg_s = _dense_soft(Ls, Ms, Vs)
    # add the identity-edge message with its softmax weight (last column)
    addm = jnp.where(M0[:, None] > 0, 0.0, NEG)
    am = jnp.maximum(jnp.max(L0 + addm, -1), a4)
    ex0 = jnp.sum(M0[:, None] * jnp.exp(L0 - am[..., None]), -1)
    ex4 = jnp.exp(a4 - am)
    w4 = ex4 / jnp.maximum(ex0 + ex4, 1e-16)
    agg_s = agg_s + jnp.einsum('ghs,gshe->gshe', w4, vr4p)
    agg_s = agg_s.reshape(NS_S, C)

    # dst-servers (type 2): incoming rel2 (units) + rel5 (paired src-server)
    kr5p = jnp.einsum('gst,gthd->gshd', P5, kr[5])
    vr5p = jnp.einsum('gst,gthd->gshd', P5, vr[5])
    L2 = jnp.einsum('gshd,guhd->ghsu', qg[2], kr[2]) * sc[2][None, :, None, None]
    a5 = jnp.einsum('gshd,gshd->ghs', qg[2], kr5p) * sc[5][None, :, None]
    Ld = jnp.concatenate([L2, a5[..., None]], -1)
    Md = jnp.concatenate([M2, jnp.ones((G, NSPG, 1), M2.dtype)], -1)
    Vd = jnp.concatenate([vr[2], jnp.zeros((G, 1, H, D), vr[2].dtype)], 1)
    agg_d = _dense_soft(Ld, Md, Vd)
    addm = jnp.where(M2[:, None] > 0, 0.0, NEG)
    am = jnp.maximum(jnp.max(L2 + addm, -1), a5)
    ex2 = jnp.sum(M2[:, None] * jnp.exp(L2 - am[..., None]), -1)
    ex5 = jnp.exp(a5 - am)
    w5 = ex5 / jnp.maximum(ex2 + ex5, 1e-16)
    agg_d = agg_d + jnp.einsum('ghs,gshe->gshe', w5, vr5p)
    agg_d = agg_d.reshape(ND_S, C)

    out = []
    for t, agg in enumerate((agg_u, agg_s, agg_d)):
        o = jax.nn.gelu(agg, approximate=False) @ ow[t] + ob[t]
        g = jax.nn.sigmoid(sk[t])
        out.append(g * o + (1.0 - g) * xs[t])
    return out


def _fwd(x_units, x_src, x_dst, in_w, in_b, k_w, k_b, q_w, q_b, v_w, v_b,
         a_rel, m_rel, p_rel, o_w, o_b, skip, out_w, out_b,
         fcu_w1, fcu_b1, fcu_w2, fcu_b2, fcs_w1, fcs_b1, fcs_w2, fcs_b2,
         mask1, mask2, M0, M1, M2, M3, P4, P5, oh1, oh2):
    xs = [jax.nn.relu(x @ in_w[t] + in_b[t]) for t, x in enumerate((x_units, x_src, x_dst))]
    for l in range(L):
        xs = _hgt_layer(xs, k_w[l], k_b[l], q_w[l], q_b[l], v_w[l], v_b[l],
                        a_rel[l], m_rel[l], p_rel[l], o_w[l], o_b[l], skip[l],
                        M0, M1, M2, M3, P4, P5)
    xs = [xs[t] @ out_w[t] + out_b[t] for t in range(3)]
    units, srv = xs[0], xs[1]
    u = jax.nn.relu(units @ fcu_w1 + fcu_b1) @ fcu_w2 + fcu_b2
    logits1 = jnp.tanh(u[:, 0]).reshape(G, NUPG) * 10.0 + mask1
    logp1_full = jax.nn.log_softmax(logits1, axis=1)
    logp1 = jnp.sum(logp1_full * oh1, axis=1)
    p1 = jnp.exp(logp1_full)
    ent1 = -jnp.sum(p1 * jnp.where(p1 > 0, logp1_full, 0.0), axis=1)
    ud = units.reshape(G, NUPG, C); sd = srv.reshape(G, NSPG, C)
    uf = jnp.einsum('gu,guc->gc', oh1, ud)
    comb = jnp.concatenate([sd, jnp.broadcast_to(uf[:, None, :], (G, NSPG, C))], axis=-1)
    s = jax.nn.relu(comb @ fcs_w1 + fcs_b1) @ fcs_w2 + fcs_b2
    logits2 = jnp.tanh(s[..., 0]) * 10.0 + mask2
    logp2_full = jax.nn.log_softmax(logits2, axis=1)
    logp2 = jnp.sum(logp2_full * oh2, axis=1)
    p2 = jnp.exp(logp2_full)
    ent2 = -jnp.sum(p2 * jnp.where(p2 > 0, logp2_full, 0.0), axis=1)
    return logp1, logp2, ent1 + ent2


_PMAP = None
_LAST_EXEC_NS = None


def _get_pmap():
    global _PMAP
    if _PMAP is None:
        in_axes = tuple([0] * 3 + [None] * 24 + [0] * 10)
        _PMAP = jax.pmap(_fwd, in_axes=in_axes, devices=jax.devices()[:NSH])
    return _PMAP


def _build_masks(inputs):
    """Host-side index preprocessing: per-graph edge multiplicity masks."""
    M0 = np.zeros((NSH, G, NSPG, NUPG), np.float32)
    M1 = np.zeros((NSH, G, NUPG, NSPG), np.float32)
    M2 = np.zeros((NSH, G, NSPG, NUPG), np.float32)
    M3 = np.zeros((NSH, G, NUPG, NSPG), np.float32)
    P4 = np.zeros((NSH, G, NSPG, NSPG), np.float32)
    P5 = np.zeros((NSH, G, NSPG, NSPG), np.float32)
    tgt = {0: (M0, 1, 0), 1: (M1, 0, 1), 2: (M2, 2, 0), 3: (M3, 0, 2),
           4: (P4, 1, 2), 5: (P5, 2, 1)}
    for r in range(6):
        e = np.asarray(inputs['edge%d' % r]).reshape(2, NSH, G * _E_PER_G[r])
        M, dt, st = tgt[r]
        npg_d, npg_s = _TYPE_PG[dt], _TYPE_PG[st]
        for sh in range(NSH):
            src = e[0, sh] - sh * _TYPE_SHARD[st]
            dst = e[1, sh] - sh * _TYPE_SHARD[dt]
            g = dst // npg_d
            np.add.at(M[sh], (g, dst % npg_d, src % npg_s), 1.0)
    return M0, M1, M2, M3, P4, P5


def kernel(**inputs):
    global _LAST_EXEC_NS
    xu = np.asarray(inputs['x_units'], np.float32).reshape(NSH, NU_S, FIN)
    xsr = np.asarray(inputs['x_src'], np.float32).reshape(NSH, NS_S, FIN)
    xds = np.asarray(inputs['x_dst'], np.float32).reshape(NSH, ND_S, FIN)
    m1 = np.asarray(inputs['mask1'], np.float32).reshape(NSH, G, NUPG)
    m2 = np.asarray(inputs['mask2'], np.float32).reshape(NSH, G, NSPG)
    a1 = np.asarray(inputs['act1'], np.int32)
    a2 = np.asarray(inputs['act2'], np.int32)
    oh1 = np.eye(NUPG, dtype=np.float32)[a1].reshape(NSH, G, NUPG)
    oh2 = np.eye(NSPG, dtype=np.float32)[a2].reshape(NSH, G, NSPG)
    M0, M1, M2, M3, P4, P5 = _build_masks(inputs)
    params = [np.asarray(inputs[n], np.float32) for n in (
        'in_w', 'in_b', 'k_w', 'k_b', 'q_w', 'q_b', 'v_w', 'v_b',
        'a_rel', 'm_rel', 'p_rel', 'o_w', 'o_b', 'skip', 'out_w', 'out_b',
        'fcu_w1', 'fcu_b1', 'fcu_w2', 'fcu_b2',
        'fcs_w1', 'fcs_b1', 'fcs_w2', 'fcs_b2')]
    args = [xu, xsr, xds] + params + [m1, m2, M0, M1, M2, M3, P4, P5, oh1, oh2]
    f = _get_pmap()
    lp1, lp2, ent = f(*args)                     # compile + warm
    jax.block_until_ready((lp1, lp2, ent))
    t0 = time.perf_counter_ns()
    lp1, lp2, ent = f(*args)
    jax.block_until_ready((lp1, lp2, ent))
    _LAST_EXEC_NS = time.perf_counter_ns() - t0
    lp1 = np.asarray(lp1).reshape(BS); lp2 = np.asarray(lp2).reshape(BS)
    ent = np.asarray(ent).reshape(BS)
    acts = np.stack([a1, a2])
    return acts, np.stack([lp1, lp2]), ent
